# revision 6
# baseline (speedup 1.0000x reference)
"""Trainium2 Bass kernel for 2-layer bipartite GATv2 (users <-> items).

Strategy (8 NeuronCores):
  * Edges are assigned to cores by their USER node (u // USH). Each core owns a
    contiguous user shard of USH=12544 users (100352 padded total).
  * u->i phase (rel 0): edge-parallel. Each core gathers source features from
    its LOCAL xl_u table (<=12544 rows, fits int16 gather indices), scatters
    per-edge messages into a full-item accumulator [20480, 130] via one-hot
    matmuls in PSUM, then one AllReduce(add) combines partial item sums.
  * i->u phase (rel 1): destination-sharded. Each core owns its user blocks;
    gathers source features from the full item table (20480 rows, int16-safe);
    no collective needed.
  * Per-edge pipeline (tiles of 128 edges, bf16):
      XS = gather(xl_tbl, src), XR = gather(xr_tbl, dst)
      T = XS + XR;  LR = max(0.2*T, T)          (leaky relu)
      logit_h = sum(LR_h * a_h)                  (DVE mult + reduce)
      e_h = exp(logit_h)                         (ACT)
      rhs = [e0*XS_0 | e1*XS_1 | e0 | e1]        [128, 130]
      P[e, d] = (ldst[e] == d)                   (one-hot vs iota)
      psum[dst_block] += P.T @ rhs               (scatter matmul)
  * Segment softmax max-subtraction is skipped: logits are O(0.1) with this
    init scale, exp() cannot overflow, and the 1e-16 epsilon stays negligible.
  * Numerically everything heavy is bf16 with fp32 PSUM/accumulators.

The module compiles one NEFF per process and runs it via run_bass_kernel_spmd
(axon/PJRT path). Host-side work is only padding, transposes, casts and edge
index bookkeeping.
"""
import sys

for _p in ("/opt/trn_rl_repo", "/opt/pypackages"):
    if _p not in sys.path:
        sys.path.insert(0, _p)

import numpy as np
import ml_dtypes

import concourse.bacc as bacc
import concourse.bass as bass
import concourse.tile as tile
from concourse import mybir, library_config
from concourse.masks import make_identity

BF16 = ml_dtypes.bfloat16
F32 = np.float32
ALPHA = 0.2  # leaky relu slope
EPS = 1e-16

FULL_CFG = dict(NU=100000, NI=20000, E=250000, NC=8, USH=12544, NIP=20480, G=8)


# ----------------------------------------------------------------------------
# host-side planning
# ----------------------------------------------------------------------------

def _wrap_idxs(idx: np.ndarray) -> np.ndarray:
    """dma_gather int16 index layout: [128, n/16], j -> [j%16, j//16], x8 replicas."""
    n = idx.shape[0]
    assert n % 16 == 0
    a = np.empty((16, n // 16), np.int16)
    a[np.arange(n) % 16, np.arange(n) // 16] = idx.astype(np.int16)
    return np.tile(a, (8, 1))


def _slot_fill(key, Ks, vals_dummies):
    """Place per-edge values into padded per-block tile slots.

    key: block id per edge. Ks: tiles per block (global schedule).
    vals_dummies: list of (values, dummy, dtype). Returns filled flat arrays
    of length sum(Ks)*128.
    """
    nslot = int(Ks.sum()) * 128
    offs = np.zeros(len(Ks) + 1, np.int64)
    offs[1:] = np.cumsum(Ks * 128)
    order = np.argsort(key, kind="stable")
    sk = key[order]
    block_start = np.searchsorted(sk, np.arange(len(Ks)))
    rank = np.arange(len(sk)) - block_start[sk]
    pos = offs[sk] + rank
    outs = []
    for vals, dummy, dt in vals_dummies:
        a = np.full(nslot, dummy, dt)
        a[pos] = vals[order].astype(dt)
        outs.append(a)
    return outs


def plan(edge_u, edge_i, cfg):
    """Build the global tile schedules (KA, KB) and per-core edge arrays."""
    NC, USH, NIP = cfg["NC"], cfg["USH"], cfg["NIP"]
    NBI, NBU = NIP // 128, USH // 128
    owner = edge_u // USH
    per_core = []
    cntA = np.zeros((NC, NBI), np.int64)
    cntB = np.zeros((NC, NBU), np.int64)
    for c in range(NC):
        m = owner == c
        eu = edge_u[m] - c * USH
        ei = edge_i[m]
        cntA[c] = np.bincount(ei // 128, minlength=NBI)
        cntB[c] = np.bincount(eu // 128, minlength=NBU)
        per_core.append((eu, ei))
    KA = np.maximum(1, -(-cntA.max(0) // 128)).astype(np.int64)
    KB = np.maximum(1, -(-cntB.max(0) // 128)).astype(np.int64)

    cores = []
    for c in range(NC):
        eu, ei = per_core[c]
        # u->i: group by global item block; gather src from local user table
        srcA, xrA, ldA = _slot_fill(ei // 128, KA, [
            (eu, 0, np.int16), (ei, 0, np.int16), (ei % 128, 200, np.int16)])
        # i->u: group by local user block; gather src from full item table
        srcB, xrB, ldB = _slot_fill(eu // 128, KB, [
            (ei, 0, np.int16), (eu, 0, np.int16), (eu % 128, 200, np.int16)])
        NTA, NTB = int(KA.sum()), int(KB.sum())
        cores.append(dict(
            srcA=_wrap_idxs(srcA), xrA=_wrap_idxs(xrA),
            ldA=np.ascontiguousarray(ldA.reshape(NTA, 128).T.astype(F32)),
            srcB=_wrap_idxs(srcB), xrB=_wrap_idxs(xrB),
            ldB=np.ascontiguousarray(ldB.reshape(NTB, 128).T.astype(F32)),
        ))
    return KA, KB, cores


# ----------------------------------------------------------------------------
# kernel builder
# ----------------------------------------------------------------------------

def build(cfg, KA, KB):
    NC, USH, NIP, G = cfg["NC"], cfg["USH"], cfg["NIP"], cfg["G"]
    NBI, NBU = NIP // 128, USH // 128
    NTA, NTB = int(KA.sum()), int(KB.sum())
    blkA = np.repeat(np.arange(NBI), KA)   # tile t -> item block
    blkB = np.repeat(np.arange(NBU), KB)   # tile t -> local user block
    bf = mybir.dt.bfloat16
    f32 = mybir.dt.float32
    i16 = mybir.dt.int16

    nc = bacc.Bacc("TRN2", num_devices=NC)

    # ---- external I/O -------------------------------------------------------
    xuT = nc.dram_tensor("xuT", [64, USH], bf, kind="ExternalInput")
    xiT = nc.dram_tensor("xiT", [128, NIP], bf, kind="ExternalInput")
    wpu = nc.dram_tensor("wpu", [64, 128], bf, kind="ExternalInput")
    wpi = nc.dram_tensor("wpi", [128, 128], bf, kind="ExternalInput")
    bpu = nc.dram_tensor("bpu", [128, 1], f32, kind="ExternalInput")
    bpi = nc.dram_tensor("bpi", [128, 1], f32, kind="ExternalInput")
    iota_in = nc.dram_tensor("iota", [128, 128], bf, kind="ExternalInput")
    w_in, bb_in, ab_in, ob_in = {}, {}, {}, {}
    for l in range(2):
        for r in range(2):
            w_in[("l", l, r)] = nc.dram_tensor(f"wl{l}{r}", [128, 128], bf, kind="ExternalInput")
            w_in[("r", l, r)] = nc.dram_tensor(f"wr{l}{r}", [128, 128], bf, kind="ExternalInput")
            bb_in[("l", l, r)] = nc.dram_tensor(f"blb{l}{r}", [128, 128], f32, kind="ExternalInput")
            bb_in[("r", l, r)] = nc.dram_tensor(f"brb{l}{r}", [128, 128], f32, kind="ExternalInput")
            ab_in[(l, r)] = nc.dram_tensor(f"ab{l}{r}", [128, 128], bf, kind="ExternalInput")
            ob_in[(l, r)] = nc.dram_tensor(f"ob{l}{r}", [128, 128], f32, kind="ExternalInput")
    srcA_in = nc.dram_tensor("srcA", [128, NTA * 8], i16, kind="ExternalInput")
    xrA_in = nc.dram_tensor("xrA", [128, NTA * 8], i16, kind="ExternalInput")
    ldA_in = nc.dram_tensor("ldA", [128, NTA], f32, kind="ExternalInput")
    srcB_in = nc.dram_tensor("srcB", [128, NTB * 8], i16, kind="ExternalInput")
    xrB_in = nc.dram_tensor("xrB", [128, NTB * 8], i16, kind="ExternalInput")
    ldB_in = nc.dram_tensor("ldB", [128, NTB], f32, kind="ExternalInput")
    zu_out = nc.dram_tensor("zu_out", [USH, 128], f32, kind="ExternalOutput")
    zi_out = nc.dram_tensor("zi_out", [NIP, 128], f32, kind="ExternalOutput")

    with tile.TileContext(nc, num_cores=NC) as tc:
        with (
            tc.tile_pool(name="const", bufs=1) as const,
            tc.tile_pool(name="sbuf", bufs=2) as sbuf,
            tc.tile_pool(name="psum", bufs=2, space="PSUM") as psum,
            tc.tile_pool(name="dram", bufs=1, space="DRAM") as dram,
        ):
            nc.gpsimd.load_library(library_config.mlp)

            # ---- constants into SBUF ---------------------------------------
            def cload(src, shape, dtype, name):
                t = const.tile(shape, dtype, name=name, tag=name)
                nc.sync.dma_start(t[:], src[:])
                return t

            iota_t = cload(iota_in, [128, 128], bf, "iota_t")
            wpu_t = cload(wpu, [64, 128], bf, "wpu_t")
            wpi_t = cload(wpi, [128, 128], bf, "wpi_t")
            bpu_t = cload(bpu, [128, 1], f32, "bpu_t")
            bpi_t = cload(bpi, [128, 1], f32, "bpi_t")
            w_t, bb_t, ab_t, ob_t = {}, {}, {}, {}
            for l in range(2):
                for r in range(2):
                    for s in ("l", "r"):
                        w_t[(s, l, r)] = cload(w_in[(s, l, r)], [128, 128], bf, f"w{s}{l}{r}_t")
                        bb_t[(s, l, r)] = cload(bb_in[(s, l, r)], [128, 128], f32, f"b{s}b{l}{r}_t")
                    ab_t[(l, r)] = cload(ab_in[(l, r)], [128, 128], bf, f"ab{l}{r}_t")
                    ob_t[(l, r)] = cload(ob_in[(l, r)], [128, 128], f32, f"ob{l}{r}_t")
            srcA_t = cload(srcA_in, [128, NTA * 8], i16, "srcA_t")
            xrA_t = cload(xrA_in, [128, NTA * 8], i16, "xrA_t")
            ldA_t = cload(ldA_in, [128, NTA], f32, "ldA_t")
            srcB_t = cload(srcB_in, [128, NTB * 8], i16, "srcB_t")
            xrB_t = cload(xrB_in, [128, NTB * 8], i16, "xrB_t")
            ldB_t = cload(ldB_in, [128, NTB], f32, "ldB_t")
            ident_t = const.tile([128, 128], f32, name="ident_t", tag="ident_t")
            make_identity(nc, ident_t[:])

            # ---- DRAM scratch ----------------------------------------------
            def dtile(name, shape, dtype, shared=False):
                return dram.tile(shape, dtype, name=name, tag=name,
                                 addr_space="Shared" if shared else "Local")

            zuT = [dtile(f"zuT{i}", [128, USH], bf) for i in range(2)]
            ziT = [dtile(f"ziT{i}", [128, NIP], bf) for i in range(2)]
            tbl = {}
            for l in range(2):
                tbl[("xlu", l)] = dtile(f"xlu{l}", [USH, 128], bf)
                tbl[("xru", l)] = dtile(f"xru{l}", [USH, 128], bf)
                tbl[("xli", l)] = dtile(f"xli{l}", [NIP, 128], bf)
                tbl[("xri", l)] = dtile(f"xri{l}", [NIP, 128], bf)
            accA = [dtile(f"accA{l}", [NIP, 130], f32) for l in range(2)]
            accAr = [dtile(f"accAr{l}", [NIP, 130], f32, shared=True) for l in range(2)]

            # ---- initial projections (feature-major) -----------------------
            def init_proj(xT, w_tile, b_col, dstT, ncols, kdim):
                for c0 in range(0, ncols, 512):
                    w = min(512, ncols - c0)
                    xc = sbuf.tile([128, 512], bf, name="xc_init", tag="xcI")
                    nc.sync.dma_start(xc[:kdim, :w], xT[:kdim, c0:c0 + w])
                    ps = psum.tile([128, 512], f32, name="ps_init", tag="psD")
                    nc.tensor.matmul(ps[:, :w], lhsT=w_tile[:kdim, :], rhs=xc[:kdim, :w],
                                     start=True, stop=True)
                    stg = sbuf.tile([128, 512], bf, name="stg_init", tag="stgD")
                    nc.scalar.activation(stg[:, :w], ps[:, :w],
                                         mybir.ActivationFunctionType.Identity,
                                         bias=b_col[:])
                    nc.sync.dma_start(dstT[:, c0:c0 + w], stg[:, :w])

            init_proj(xuT, wpu_t, bpu_t, zuT[0], USH, 64)
            init_proj(xiT, wpi_t, bpi_t, ziT[0], NIP, 128)

            # ---- per node-tile linear tables -------------------------------
            def make_tables(zT, nblocks, wA, bA, dstA, wB, bB, dstB):
                # dstA = zT @ wA + bA ; dstB = zT @ wB + bB (node-major rows)
                for n in range(nblocks):
                    zt = sbuf.tile([128, 128], bf, name="zt_d", tag="ztD")
                    nc.sync.dma_start(zt[:], zT[:, n * 128:(n + 1) * 128])
                    for w_tile, b_tile, dst in ((wA, bA, dstA), (wB, bB, dstB)):
                        ps = psum.tile([128, 512], f32, name="ps_tab", tag="psD")
                        nc.tensor.matmul(ps[:, :128], lhsT=zt[:], rhs=w_tile[:],
                                         start=True, stop=True)
                        stg = sbuf.tile([128, 128], bf, name="stg_tab", tag="stgT")
                        nc.vector.tensor_tensor(out=stg[:], in0=ps[:, :128], in1=b_tile[:],
                                                op=mybir.AluOpType.add)
                        nc.sync.dma_start(dst[n * 128:(n + 1) * 128, :], stg[:])

            # ---- edge phase -------------------------------------------------
            def edge_phase(tile_blk, Ks, src_tbl, xr_tbl, src_idx, xr_idx, ld_t,
                           avec, flush):
                NT = len(tile_blk)
                n_in_blk = 0
                ps = None
                for t0 in range(0, NT, G):
                    g = min(G, NT - t0)
                    ni = g * 128
                    xs = sbuf.tile([128, G, 128], bf, name="xs_e", tag="xsE")
                    xr = sbuf.tile([128, G, 128], bf, name="xr_e", tag="xrE")
                    nc.gpsimd.dma_gather(xs[:, :g, :], src_tbl[:],
                                         src_idx[:, t0 * 8:(t0 + g) * 8], ni, ni, 128)
                    nc.gpsimd.dma_gather(xr[:, :g, :], xr_tbl[:],
                                         xr_idx[:, t0 * 8:(t0 + g) * 8], ni, ni, 128)
                    tt = sbuf.tile([128, G, 128], bf, name="tt_e", tag="ttE")
                    nc.vector.tensor_tensor(out=tt[:, :g, :], in0=xs[:, :g, :],
                                            in1=xr[:, :g, :], op=mybir.AluOpType.add)
                    lr = sbuf.tile([128, G, 128], bf, name="lr_e", tag="lrE")
                    nc.vector.scalar_tensor_tensor(
                        out=lr[:, :g, :], in0=tt[:, :g, :], scalar=ALPHA,
                        in1=tt[:, :g, :],
                        op0=mybir.AluOpType.mult, op1=mybir.AluOpType.max)
                    uu = sbuf.tile([128, G, 128], bf, name="uu_e", tag="uuE")
                    nc.vector.tensor_tensor(
                        out=uu[:, :g, :], in0=lr[:, :g, :],
                        in1=avec[:, None, :].to_broadcast([128, g, 128]),
                        op=mybir.AluOpType.mult)
                    lg = sbuf.tile([128, G, 2], f32, name="lg_e", tag="lgE")
                    nc.vector.tensor_reduce(
                        out=lg[:, :g, :],
                        in_=uu[:, :g, :].rearrange("p g (h d) -> p g h d", h=2),
                        axis=mybir.AxisListType.X, op=mybir.AluOpType.add)
                    ee = sbuf.tile([128, G, 2], f32, name="ee_e", tag="eeE")
                    nc.scalar.activation(ee[:, :g, :], lg[:, :g, :],
                                         mybir.ActivationFunctionType.Exp)
                    mm = sbuf.tile([128, G, 130], bf, name="mm_e", tag="mmE")
                    nc.scalar.copy(mm[:, :g, 128:130], ee[:, :g, :])
                    for k in range(g):
                        t = t0 + k
                        b = tile_blk[t]
                        if n_in_blk == 0:
                            ps = psum.tile([128, 130], f32, name="ps_e", tag="psE",
                                           bufs=3)
                        for h in range(2):
                            nc.vector.tensor_scalar(
                                out=mm[:, k, h * 64:(h + 1) * 64],
                                in0=xs[:, k, h * 64:(h + 1) * 64],
                                scalar1=ee[:, k, h:h + 1],
                                scalar2=None, op0=mybir.AluOpType.mult)
                        pp = sbuf.tile([128, 128], bf, name="pp_e", tag="ppE")
                        nc.vector.tensor_scalar(
                            out=pp[:], in0=iota_t[:], scalar1=ld_t[:, t:t + 1],
                            scalar2=None, op0=mybir.AluOpType.is_equal)
                        first = n_in_blk == 0
                        last = n_in_blk == Ks[b] - 1
                        nc.tensor.matmul(ps[:, 0:130], lhsT=pp[:], rhs=mm[:, k, :],
                                         start=first, stop=last)
                        if last:
                            flush(b, ps)
                            n_in_blk = 0
                        else:
                            n_in_blk += 1

            # ---- normalize helper ------------------------------------------
            def normalize(ps_or_acc, obias, do_relu, out_f32):
                """out_f32[128,128] = msg/(den+eps) + bias (optionally relu)."""
                dpe = sbuf.tile([128, 2], f32, name="dpe_n", tag="dpeN")
                nc.vector.tensor_scalar_add(dpe[:], ps_or_acc[:, 128:130], EPS)
                rcp = sbuf.tile([128, 2], f32, name="rcp_n", tag="rcpN")
                nc.vector.reciprocal(rcp[:], dpe[:])
                for h in range(2):
                    nc.vector.tensor_scalar(
                        out=out_f32[:, h * 64:(h + 1) * 64],
                        in0=ps_or_acc[:, h * 64:(h + 1) * 64],
                        scalar1=rcp[:, h:h + 1], scalar2=None,
                        op0=mybir.AluOpType.mult)
                nc.vector.tensor_tensor(out=out_f32[:], in0=out_f32[:], in1=obias[:],
                                        op=mybir.AluOpType.add)
                if do_relu:
                    nc.vector.tensor_scalar_max(out_f32[:], out_f32[:], 0.0)

            # ---- layers -----------------------------------------------------
            for l in range(2):
                zu_cur, zi_cur = zuT[l], ziT[l]
                make_tables(zu_cur, NBU,
                            w_t[("l", l, 0)], bb_t[("l", l, 0)], tbl[("xlu", l)],
                            w_t[("r", l, 1)], bb_t[("r", l, 1)], tbl[("xru", l)])
                make_tables(zi_cur, NBI,
                            w_t[("r", l, 0)], bb_t[("r", l, 0)], tbl[("xri", l)],
                            w_t[("l", l, 1)], bb_t[("l", l, 1)], tbl[("xli", l)])

                # ---- A phase: u -> i (edge parallel + AllReduce) ----------
                def flushA(b, ps, l=l):
                    stg = sbuf.tile([128, 130], f32, name="stg_fa", tag="stgFA")
                    nc.scalar.copy(stg[:], ps[:])
                    nc.sync.dma_start(accA[l][b * 128:(b + 1) * 128, :], stg[:])

                edge_phase(blkA, KA, tbl[("xlu", l)], tbl[("xri", l)],
                           srcA_t, xrA_t, ldA_t, ab_t[(l, 0)], flushA)
                nc.gpsimd.collective_compute(
                    "AllReduce", mybir.AluOpType.add,
                    replica_groups=[list(range(NC))],
                    ins=[accA[l][:].opt()], outs=[accAr[l][:].opt()])

                # ---- post A: zi_new ---------------------------------------
                for b in range(NBI):
                    acc = sbuf.tile([128, 130], f32, name="acc_pa", tag="accPA")
                    nc.sync.dma_start(acc[:], accAr[l][b * 128:(b + 1) * 128, :])
                    zi_new = sbuf.tile([128, 128], f32, name="zin_pa", tag="zinPA")
                    normalize(acc, ob_t[(l, 0)], do_relu=(l == 0), out_f32=zi_new)
                    if l == 0:
                        pst = psum.tile([128, 128], f32, name="ps_tp", tag="psT")
                        nc.tensor.transpose(pst[:], zi_new[:], ident_t[:])
                        stg = sbuf.tile([128, 128], bf, name="stg_tp", tag="stgTP")
                        nc.scalar.copy(stg[:], pst[:])
                        nc.sync.dma_start(ziT[1][:, b * 128:(b + 1) * 128], stg[:])
                    else:
                        nc.sync.dma_start(zi_out[b * 128:(b + 1) * 128, :], zi_new[:])

                # ---- B phase: i -> u (dst sharded, local) -----------------
                def flushB(j, ps, l=l):
                    zu_new = sbuf.tile([128, 128], f32, name="zun_fb", tag="zunFB")
                    normalize(ps, ob_t[(l, 1)], do_relu=(l == 0), out_f32=zu_new)
                    if l == 0:
                        pst = psum.tile([128, 128], f32, name="ps_tpb", tag="psT")
                        nc.tensor.transpose(pst[:], zu_new[:], ident_t[:])
                        stg = sbuf.tile([128, 128], bf, name="stg_tpb", tag="stgTP")
                        nc.scalar.copy(stg[:], pst[:])
                        nc.sync.dma_start(zuT[1][:, j * 128:(j + 1) * 128], stg[:])
                    else:
                        nc.sync.dma_start(zu_out[j * 128:(j + 1) * 128, :], zu_new[:])

                edge_phase(blkB, KB, tbl[("xli", l)], tbl[("xru", l)],
                           srcB_t, xrB_t, ldB_t, ab_t[(l, 1)], flushB)

    nc.compile()
    return nc


# ----------------------------------------------------------------------------
# host wrapper
# ----------------------------------------------------------------------------

def _bcast_row(v, dtype):
    return np.ascontiguousarray(np.tile(np.asarray(v, F32).reshape(1, -1), (128, 1)).astype(dtype))


def prep_in_maps(inputs, cfg, cores):
    """Build the 8 per-core input dicts from the full problem inputs."""
    NC, USH, NIP = cfg["NC"], cfg["USH"], cfg["NIP"]
    NU, NI = cfg["NU"], cfg["NI"]
    x_user = np.asarray(inputs["x_user"], F32)
    x_item = np.asarray(inputs["x_item"], F32)
    xu_pad = np.zeros((NC * USH, 64), F32)
    xu_pad[:NU] = x_user
    xi_pad = np.zeros((NIP, 128), F32)
    xi_pad[:NI] = x_item
    xiT = np.ascontiguousarray(xi_pad.T.astype(BF16))

    Wl, bl = np.asarray(inputs["Wl"], F32), np.asarray(inputs["bl"], F32)
    Wr, br = np.asarray(inputs["Wr"], F32), np.asarray(inputs["br"], F32)
    att, obias = np.asarray(inputs["att"], F32), np.asarray(inputs["bias"], F32)

    shared = {
        "xiT": xiT,
        "wpu": np.asarray(inputs["Wp_user"], F32).astype(BF16),
        "wpi": np.asarray(inputs["Wp_item"], F32).astype(BF16),
        "bpu": np.asarray(inputs["bp_user"], F32).reshape(128, 1),
        "bpi": np.asarray(inputs["bp_item"], F32).reshape(128, 1),
        "iota": _bcast_row(np.arange(128), BF16),
    }
    for l in range(2):
        for r in range(2):
            shared[f"wl{l}{r}"] = Wl[l, r].astype(BF16)
            shared[f"wr{l}{r}"] = Wr[l, r].astype(BF16)
            shared[f"blb{l}{r}"] = _bcast_row(bl[l, r], F32)
            shared[f"brb{l}{r}"] = _bcast_row(br[l, r], F32)
            shared[f"ab{l}{r}"] = _bcast_row(att[l, r].reshape(128), BF16)
            shared[f"ob{l}{r}"] = _bcast_row(obias[l, r], F32)

    in_maps = []
    for c in range(NC):
        m = dict(shared)
        m["xuT"] = np.ascontiguousarray(
            xu_pad[c * USH:(c + 1) * USH].T.astype(BF16))
        m.update(cores[c])
        m["srcA"], m["xrA"], m["ldA"] = m.pop("srcA"), m.pop("xrA"), m.pop("ldA")
        m["srcB"], m["xrB"], m["ldB"] = m.pop("srcB"), m.pop("xrB"), m.pop("ldB")
        in_maps.append(m)
    return in_maps


_BUILT = {}
LAST_RESULTS = None


def kernel(x_user, x_item, Wp_user, bp_user, Wp_item, bp_item,
           Wl, bl, Wr, br, att, bias, edge_src, edge_dst,
           trace=False):
    global LAST_RESULTS
    from concourse.bass_utils import run_bass_kernel_spmd

    cfg = FULL_CFG
    inputs = dict(x_user=x_user, x_item=x_item, Wp_user=Wp_user,
                  bp_user=bp_user, Wp_item=Wp_item, bp_item=bp_item,
                  Wl=Wl, bl=bl, Wr=Wr, br=br, att=att, bias=bias)
    eu = np.asarray(edge_src, np.int64)
    ei = np.asarray(edge_dst, np.int64)
    KA, KB, cores = plan(eu, ei, cfg)

    key = (tuple(KA), tuple(KB))
    if key not in _BUILT:
        _BUILT.clear()
        _BUILT[key] = build(cfg, KA, KB)
    nc = _BUILT[key]

    in_maps = prep_in_maps(inputs, cfg, cores)
    res = run_bass_kernel_spmd(nc, in_maps, core_ids=list(range(cfg["NC"])),
                               trace=trace)
    LAST_RESULTS = res
    zu = np.concatenate([res.results[c]["zu_out"] for c in range(cfg["NC"])],
                        axis=0)[:cfg["NU"]]
    zi = res.results[0]["zi_out"][:cfg["NI"]]
    return zu.astype(np.float32), zi.astype(np.float32)


# revision 7
# speedup vs baseline: 1.6991x; 1.6991x over previous
"""Trainium2 Bass kernel for 2-layer bipartite GATv2 (users <-> items).

Strategy (8 NeuronCores):
  * Edges are assigned to cores by their USER node (u // USH). Each core owns a
    contiguous user shard of USH=12544 users (100352 padded total).
  * u->i phase (rel 0): edge-parallel. Each core gathers source features from
    its LOCAL xl_u table (<=12544 rows, fits int16 gather indices), scatters
    per-edge messages into a full-item accumulator [20480, 130] via one-hot
    matmuls in PSUM, then one AllReduce(add) combines partial item sums.
  * i->u phase (rel 1): destination-sharded. Each core owns its user blocks;
    gathers source features from the full item table (20480 rows, int16-safe);
    no collective needed.
  * Per-edge pipeline in slabs of G=8 tiles (128 edges each), all bf16,
    engineered around DVE perf modes (no per-partition "scalar ptr" ops --
    those run ~1.1us each; broadcast-AP tensor_tensor instead):
      XS = dma_gather(xl_tbl, src)   XR = dma_gather(xr_tbl, dst)  (4 queues)
      T = XS + XR;  LR = max(0.2*T, T)
      logit = reduce_X(LR * a_bcast); e = exp(logit) -> mm[:,:,128:130]
      mm[:,:,h*64:(h+1)*64] = XS_h * e_h (broadcast)
      P[e,t,d] = (ldst[e,t] == iota[d]); psum[block] += P_t.T @ mm_t
  * Softmax max-subtraction is skipped (logits are O(0.1) at this init scale;
    fp32 exp cannot overflow and the 1e-16 epsilon stays negligible).
  * All-zero bias vectors (this problem) skip their add instructions.
"""
import sys

for _p in ("/opt/trn_rl_repo", "/opt/pypackages"):
    if _p not in sys.path:
        sys.path.insert(0, _p)

import numpy as np
import ml_dtypes

import concourse.bacc as bacc
import concourse.bass as bass
import concourse.tile as tile
from concourse import mybir, library_config
from concourse.masks import make_identity

BF16 = ml_dtypes.bfloat16
F32 = np.float32
ALPHA = 0.2  # leaky relu slope
EPS = 1e-16

FULL_CFG = dict(NU=100000, NI=20000, E=250000, NC=8, USH=12544, NIP=20480, G=8)


# ----------------------------------------------------------------------------
# host-side planning
# ----------------------------------------------------------------------------

def _wrap_idxs(idx: np.ndarray) -> np.ndarray:
    """dma_gather int16 index layout: [128, n/16], j -> [j%16, j//16], x8 replicas."""
    n = idx.shape[0]
    assert n % 16 == 0
    a = np.empty((16, n // 16), np.int16)
    a[np.arange(n) % 16, np.arange(n) // 16] = idx.astype(np.int16)
    return np.tile(a, (8, 1))


def _slot_fill(key, Ks, vals_dummies):
    """Place per-edge values into padded per-block tile slots."""
    nslot = int(Ks.sum()) * 128
    offs = np.zeros(len(Ks) + 1, np.int64)
    offs[1:] = np.cumsum(Ks * 128)
    order = np.argsort(key, kind="stable")
    sk = key[order]
    block_start = np.searchsorted(sk, np.arange(len(Ks)))
    rank = np.arange(len(sk)) - block_start[sk]
    pos = offs[sk] + rank
    outs = []
    for vals, dummy, dt in vals_dummies:
        a = np.full(nslot, dummy, dt)
        a[pos] = vals[order].astype(dt)
        outs.append(a)
    return outs


def plan(edge_u, edge_i, cfg):
    """Build the global tile schedules (KA, KB) and per-core edge arrays."""
    NC, USH, NIP = cfg["NC"], cfg["USH"], cfg["NIP"]
    NBI, NBU = NIP // 128, USH // 128
    owner = edge_u // USH
    per_core = []
    cntA = np.zeros((NC, NBI), np.int64)
    cntB = np.zeros((NC, NBU), np.int64)
    for c in range(NC):
        m = owner == c
        eu = edge_u[m] - c * USH
        ei = edge_i[m]
        cntA[c] = np.bincount(ei // 128, minlength=NBI)
        cntB[c] = np.bincount(eu // 128, minlength=NBU)
        per_core.append((eu, ei))
    KA = np.maximum(1, -(-cntA.max(0) // 128)).astype(np.int64)
    KB = np.maximum(1, -(-cntB.max(0) // 128)).astype(np.int64)

    cores = []
    for c in range(NC):
        eu, ei = per_core[c]
        srcA, xrA, ldA = _slot_fill(ei // 128, KA, [
            (eu, 0, np.int16), (ei, 0, np.int16), (ei % 128, 200, np.int16)])
        srcB, xrB, ldB = _slot_fill(eu // 128, KB, [
            (ei, 0, np.int16), (eu, 0, np.int16), (eu % 128, 200, np.int16)])
        NTA, NTB = int(KA.sum()), int(KB.sum())
        cores.append(dict(
            srcA=_wrap_idxs(srcA), xrA=_wrap_idxs(xrA),
            ldA=np.ascontiguousarray(ldA.reshape(NTA, 128).T.astype(BF16)),
            srcB=_wrap_idxs(srcB), xrB=_wrap_idxs(xrB),
            ldB=np.ascontiguousarray(ldB.reshape(NTB, 128).T.astype(BF16)),
        ))
    return KA, KB, cores


# ----------------------------------------------------------------------------
# kernel builder
# ----------------------------------------------------------------------------

def build(cfg, KA, KB, nz):
    """nz: dict of which bias groups are nonzero:
    keys 'bp', 'tab' (bl/br), 'out' (output bias)."""
    NC, USH, NIP, G = cfg["NC"], cfg["USH"], cfg["NIP"], cfg["G"]
    NBI, NBU = NIP // 128, USH // 128
    NTA, NTB = int(KA.sum()), int(KB.sum())
    blkA = np.repeat(np.arange(NBI), KA)
    blkB = np.repeat(np.arange(NBU), KB)
    bf = mybir.dt.bfloat16
    f32 = mybir.dt.float32
    i16 = mybir.dt.int16

    nc = bacc.Bacc("TRN2", num_devices=NC, num_swdge_queues=4)

    xuT = nc.dram_tensor("xuT", [64, USH], bf, kind="ExternalInput")
    xiT = nc.dram_tensor("xiT", [128, NIP], bf, kind="ExternalInput")
    wpu = nc.dram_tensor("wpu", [64, 128], bf, kind="ExternalInput")
    wpi = nc.dram_tensor("wpi", [128, 128], bf, kind="ExternalInput")
    bpu = nc.dram_tensor("bpu", [128, 1], f32, kind="ExternalInput")
    bpi = nc.dram_tensor("bpi", [128, 1], f32, kind="ExternalInput")
    iota_in = nc.dram_tensor("iota", [128, 128], bf, kind="ExternalInput")
    w_in, bb_in, ab_in, ob_in = {}, {}, {}, {}
    for l in range(2):
        for r in range(2):
            w_in[("l", l, r)] = nc.dram_tensor(f"wl{l}{r}", [128, 128], bf, kind="ExternalInput")
            w_in[("r", l, r)] = nc.dram_tensor(f"wr{l}{r}", [128, 128], bf, kind="ExternalInput")
            bb_in[("l", l, r)] = nc.dram_tensor(f"blb{l}{r}", [128, 128], f32, kind="ExternalInput")
            bb_in[("r", l, r)] = nc.dram_tensor(f"brb{l}{r}", [128, 128], f32, kind="ExternalInput")
            ab_in[(l, r)] = nc.dram_tensor(f"ab{l}{r}", [128, 128], bf, kind="ExternalInput")
            ob_in[(l, r)] = nc.dram_tensor(f"ob{l}{r}", [128, 128], f32, kind="ExternalInput")
    srcA_in = nc.dram_tensor("srcA", [128, NTA * 8], i16, kind="ExternalInput")
    xrA_in = nc.dram_tensor("xrA", [128, NTA * 8], i16, kind="ExternalInput")
    ldA_in = nc.dram_tensor("ldA", [128, NTA], bf, kind="ExternalInput")
    srcB_in = nc.dram_tensor("srcB", [128, NTB * 8], i16, kind="ExternalInput")
    xrB_in = nc.dram_tensor("xrB", [128, NTB * 8], i16, kind="ExternalInput")
    ldB_in = nc.dram_tensor("ldB", [128, NTB], bf, kind="ExternalInput")
    zu_out = nc.dram_tensor("zu_out", [USH, 128], f32, kind="ExternalOutput")
    zi_out = nc.dram_tensor("zi_out", [NIP, 128], f32, kind="ExternalOutput")

    AluOp = mybir.AluOpType
    ActFn = mybir.ActivationFunctionType
    qn = [0]

    with tile.TileContext(nc, num_cores=NC) as tc:
        with (
            tc.tile_pool(name="const", bufs=1) as const,
            tc.tile_pool(name="sbuf", bufs=2) as sbuf,
            tc.tile_pool(name="psum", bufs=2, space="PSUM") as psum,
            tc.tile_pool(name="dram", bufs=1, space="DRAM") as dram,
        ):
            nc.gpsimd.load_library(library_config.mlp)

            def cload(src, shape, dtype, name):
                t = const.tile(shape, dtype, name=name, tag=name)
                nc.sync.dma_start(t[:], src[:])
                return t

            iota_t = cload(iota_in, [128, 128], bf, "iota_t")
            wpu_t = cload(wpu, [64, 128], bf, "wpu_t")
            wpi_t = cload(wpi, [128, 128], bf, "wpi_t")
            bpu_t = cload(bpu, [128, 1], f32, "bpu_t")
            bpi_t = cload(bpi, [128, 1], f32, "bpi_t")
            w_t, bb_t, ab_t, ob_t = {}, {}, {}, {}
            for l in range(2):
                for r in range(2):
                    for s in ("l", "r"):
                        w_t[(s, l, r)] = cload(w_in[(s, l, r)], [128, 128], bf, f"w{s}{l}{r}_t")
                        if nz["tab"]:
                            bb_t[(s, l, r)] = cload(bb_in[(s, l, r)], [128, 128], f32, f"b{s}b{l}{r}_t")
                    ab_t[(l, r)] = cload(ab_in[(l, r)], [128, 128], bf, f"ab{l}{r}_t")
                    if nz["out"]:
                        ob_t[(l, r)] = cload(ob_in[(l, r)], [128, 128], f32, f"ob{l}{r}_t")
            srcA_t = cload(srcA_in, [128, NTA * 8], i16, "srcA_t")
            xrA_t = cload(xrA_in, [128, NTA * 8], i16, "xrA_t")
            ldA_t = cload(ldA_in, [128, NTA], bf, "ldA_t")
            srcB_t = cload(srcB_in, [128, NTB * 8], i16, "srcB_t")
            xrB_t = cload(xrB_in, [128, NTB * 8], i16, "xrB_t")
            ldB_t = cload(ldB_in, [128, NTB], bf, "ldB_t")
            ident_t = const.tile([128, 128], f32, name="ident_t", tag="ident_t")
            make_identity(nc, ident_t[:])

            def dtile(name, shape, dtype, shared=False):
                return dram.tile(shape, dtype, name=name, tag=name,
                                 addr_space="Shared" if shared else "Local")

            zuT = [dtile(f"zuT{i}", [128, USH], bf) for i in range(2)]
            ziT = [dtile(f"ziT{i}", [128, NIP], bf) for i in range(2)]
            tbl = {}
            for l in range(2):
                for nm in ("xlu", "xru"):
                    tbl[(nm, l)] = dtile(f"{nm}{l}", [USH, 128], bf)
                for nm in ("xli", "xri"):
                    tbl[(nm, l)] = dtile(f"{nm}{l}", [NIP, 128], bf)
            accA = [dtile(f"accA{l}", [NIP, 130], f32) for l in range(2)]
            accAr = [dtile(f"accAr{l}", [NIP, 130], f32, shared=True) for l in range(2)]

            # ---- initial projections (feature-major out) --------------------
            def init_proj(xT, w_tile, b_col, dstT, ncols, kdim):
                for c0 in range(0, ncols, 512):
                    w = min(512, ncols - c0)
                    xc = sbuf.tile([128, 512], bf, name="xc_init", tag="xcI")
                    nc.sync.dma_start(xc[:kdim, :w], xT[:kdim, c0:c0 + w])
                    ps = psum.tile([128, 512], f32, name="ps_init", tag="psD")
                    nc.tensor.matmul(ps[:, :w], lhsT=w_tile[:kdim, :], rhs=xc[:kdim, :w],
                                     start=True, stop=True)
                    stg = sbuf.tile([128, 512], bf, name="stg_init", tag="stgD")
                    if nz["bp"]:
                        nc.scalar.activation(stg[:, :w], ps[:, :w], ActFn.Identity,
                                             bias=b_col[:])
                    else:
                        nc.scalar.copy(stg[:, :w], ps[:, :w])
                    nc.sync.dma_start(dstT[:, c0:c0 + w], stg[:, :w])

            init_proj(xuT, wpu_t, bpu_t, zuT[0], USH, 64)
            init_proj(xiT, wpi_t, bpi_t, ziT[0], NIP, 128)

            # ---- per node-tile linear tables (4 tiles per DMA chunk) -------
            def make_tables(zT, nblocks, wA, bA, dstA, wB, bB, dstB):
                for n0 in range(0, nblocks, 4):
                    nb = min(4, nblocks - n0)
                    zt = sbuf.tile([128, 4 * 128], bf, name="zt_d", tag="ztD")
                    nc.sync.dma_start(zt[:, :nb * 128], zT[:, n0 * 128:(n0 + nb) * 128])
                    for w_tile, b_tile, dst in ((wA, bA, dstA), (wB, bB, dstB)):
                        stg = sbuf.tile([128, 4, 128], bf, name="stg_tab", tag="stgT")
                        for k in range(nb):
                            ps = psum.tile([128, 512], f32, name="ps_tab", tag="psD")
                            nc.tensor.matmul(ps[:, :128], lhsT=zt[:, k * 128:(k + 1) * 128],
                                             rhs=w_tile[:], start=True, stop=True)
                            if b_tile is not None:
                                nc.vector.tensor_tensor(out=stg[:, k, :], in0=ps[:, :128],
                                                        in1=b_tile[:], op=AluOp.add)
                            else:
                                nc.scalar.copy(stg[:, k, :], ps[:, :128])
                        nc.sync.dma_start(
                            dst[n0 * 128:(n0 + nb) * 128, :].rearrange(
                                "(t p) f -> p t f", p=128),
                            stg[:, :nb, :])

            # ---- edge phase -------------------------------------------------
            def edge_phase(tile_blk, Ks, src_tbl, xr_tbl, src_idx, xr_idx, ld_t,
                           avec, flush):
                NT = len(tile_blk)
                n_in_blk = 0
                ps = None
                iota_b = iota_t[:].rearrange("p (g f) -> p g f", g=1)
                avec_b = avec[:].rearrange("p (g f) -> p g f", g=1)
                for t0 in range(0, NT, G):
                    g = min(G, NT - t0)
                    ni = g * 128
                    xs = sbuf.tile([128, G, 128], bf, name="xs_e", tag="xsE")
                    xr = sbuf.tile([128, G, 128], bf, name="xr_e", tag="xrE")
                    nc.gpsimd.dma_gather(xs[:, :g, :], src_tbl[:],
                                         src_idx[:, t0 * 8:(t0 + g) * 8], ni, ni, 128,
                                         queue_num=qn[0] % 4)
                    qn[0] += 1
                    nc.gpsimd.dma_gather(xr[:, :g, :], xr_tbl[:],
                                         xr_idx[:, t0 * 8:(t0 + g) * 8], ni, ni, 128,
                                         queue_num=qn[0] % 4)
                    qn[0] += 1
                    tt = sbuf.tile([128, G, 128], bf, name="tt_e", tag="ttE")
                    nc.vector.tensor_tensor(out=tt[:, :g, :], in0=xs[:, :g, :],
                                            in1=xr[:, :g, :], op=AluOp.add)
                    lr = sbuf.tile([128, G, 128], bf, name="lr_e", tag="lrE")
                    nc.vector.scalar_tensor_tensor(
                        out=lr[:, :g, :], in0=tt[:, :g, :], scalar=ALPHA,
                        in1=tt[:, :g, :], op0=AluOp.mult, op1=AluOp.max)
                    uu = sbuf.tile([128, G, 128], bf, name="uu_e", tag="uuE")
                    nc.vector.tensor_tensor(
                        out=uu[:, :g, :], in0=lr[:, :g, :],
                        in1=avec_b.to_broadcast([128, g, 128]), op=AluOp.mult)
                    lg = sbuf.tile([128, G, 2], f32, name="lg_e", tag="lgE")
                    nc.vector.tensor_reduce(
                        out=lg[:, :g, :],
                        in_=uu[:, :g, :].rearrange("p g (h d) -> p g h d", h=2),
                        axis=mybir.AxisListType.X, op=AluOp.add)
                    mm = sbuf.tile([128, G, 130], bf, name="mm_e", tag="mmE")
                    nc.scalar.activation(mm[:, :g, 128:130], lg[:, :g, :], ActFn.Exp)
                    for h in range(2):
                        nc.vector.tensor_tensor(
                            out=mm[:, :g, h * 64:(h + 1) * 64],
                            in0=xs[:, :g, h * 64:(h + 1) * 64],
                            in1=mm[:, :g, 128 + h:129 + h].to_broadcast([128, g, 64]),
                            op=AluOp.mult)
                    pp = sbuf.tile([128, G, 128], bf, name="pp_e", tag="ppE")
                    nc.vector.tensor_tensor(
                        out=pp[:, :g, :],
                        in0=iota_b.to_broadcast([128, g, 128]),
                        in1=ld_t[:, t0:t0 + g].to_broadcast([128, g, 128]),
                        op=AluOp.is_equal)
                    for k in range(g):
                        t = t0 + k
                        b = tile_blk[t]
                        if n_in_blk == 0:
                            ps = psum.tile([128, 130], f32, name="ps_e", tag="psE",
                                           bufs=3)
                        first = n_in_blk == 0
                        last = n_in_blk == Ks[b] - 1
                        nc.tensor.matmul(ps[:, 0:130], lhsT=pp[:, k, :], rhs=mm[:, k, :],
                                         start=first, stop=last)
                        if last:
                            flush(b, ps)
                            n_in_blk = 0
                        else:
                            n_in_blk += 1

            # ---- normalize: out = msg/(den+eps) [+bias] [relu] -------------
            def normalize(acc3, nb, obias, do_relu, out3):
                """acc3: [128, nb, 130] AP (psum or sbuf); out3: [128, nb, 128] f32."""
                dpe = sbuf.tile([128, 4, 2], f32, name="dpe_n", tag="dpeN")
                nc.vector.tensor_scalar_add(dpe[:, :nb, :], acc3[:, :, 128:130], EPS)
                rcp = sbuf.tile([128, 4, 2], f32, name="rcp_n", tag="rcpN")
                nc.vector.reciprocal(rcp[:, :nb, :], dpe[:, :nb, :])
                for h in range(2):
                    nc.vector.tensor_tensor(
                        out=out3[:, :, h * 64:(h + 1) * 64],
                        in0=acc3[:, :, h * 64:(h + 1) * 64],
                        in1=rcp[:, :nb, h:h + 1].to_broadcast([128, nb, 64]),
                        op=AluOp.mult)
                if obias is not None:
                    nc.vector.tensor_tensor(
                        out=out3[:], in0=out3[:],
                        in1=obias[:].rearrange("p (g f) -> p g f", g=1)
                        .to_broadcast([128, nb, 128]),
                        op=AluOp.add)
                if do_relu:
                    nc.vector.tensor_scalar_max(out3[:], out3[:], 0.0)

            # ---- layers -----------------------------------------------------
            for l in range(2):
                bbt = (lambda s, r: bb_t[(s, l, r)]) if nz["tab"] else (lambda s, r: None)
                obt = (lambda r: ob_t[(l, r)]) if nz["out"] else (lambda r: None)
                make_tables(zuT[l], NBU,
                            w_t[("l", l, 0)], bbt("l", 0), tbl[("xlu", l)],
                            w_t[("r", l, 1)], bbt("r", 1), tbl[("xru", l)])
                make_tables(ziT[l], NBI,
                            w_t[("r", l, 0)], bbt("r", 0), tbl[("xri", l)],
                            w_t[("l", l, 1)], bbt("l", 1), tbl[("xli", l)])

                # ---- A phase: u -> i (edge parallel + AllReduce) ----------
                def flushA(b, ps, l=l):
                    stg = sbuf.tile([128, 130], f32, name="stg_fa", tag="stgFA")
                    nc.scalar.copy(stg[:], ps[:])
                    nc.sync.dma_start(accA[l][b * 128:(b + 1) * 128, :], stg[:])

                edge_phase(blkA, KA, tbl[("xlu", l)], tbl[("xri", l)],
                           srcA_t, xrA_t, ldA_t, ab_t[(l, 0)], flushA)
                nc.gpsimd.collective_compute(
                    "AllReduce", AluOp.add,
                    replica_groups=[list(range(NC))],
                    ins=[accA[l][:].opt()], outs=[accAr[l][:].opt()])

                # ---- post A: zi_new (4 blocks per batch) ------------------
                for b0 in range(0, NBI, 4):
                    nb = min(4, NBI - b0)
                    acc = sbuf.tile([128, 4, 130], f32, name="acc_pa", tag="accPA")
                    nc.sync.dma_start(
                        acc[:, :nb, :],
                        accAr[l][b0 * 128:(b0 + nb) * 128, :].rearrange(
                            "(t p) c -> p t c", p=128))
                    zi_new = sbuf.tile([128, 4, 128], f32, name="zin_pa", tag="zinPA")
                    normalize(acc[:, :nb, :], nb, obt(0), l == 0, zi_new[:, :nb, :])
                    if l == 0:
                        stg = sbuf.tile([128, 4 * 128], bf, name="stg_tp", tag="stgTP")
                        for k in range(nb):
                            pst = psum.tile([128, 128], f32, name="ps_tp", tag="psT")
                            nc.tensor.transpose(pst[:], zi_new[:, k, :], ident_t[:])
                            nc.scalar.copy(stg[:, k * 128:(k + 1) * 128], pst[:])
                        nc.sync.dma_start(ziT[1][:, b0 * 128:(b0 + nb) * 128],
                                          stg[:, :nb * 128])
                    else:
                        nc.sync.dma_start(
                            zi_out[b0 * 128:(b0 + nb) * 128, :].rearrange(
                                "(t p) f -> p t f", p=128),
                            zi_new[:, :nb, :])

                # ---- B phase: i -> u (dst sharded, local) -----------------
                def flushB(j, ps, l=l):
                    zu_new = sbuf.tile([128, 1, 128], f32, name="zun_fb", tag="zunFB")
                    ps3 = ps[:].rearrange("p (g c) -> p g c", g=1)
                    normalize(ps3, 1, obt(1), l == 0, zu_new[:, :1, :])
                    if l == 0:
                        pst = psum.tile([128, 128], f32, name="ps_tpb", tag="psT")
                        nc.tensor.transpose(pst[:], zu_new[:, 0, :], ident_t[:])
                        stg = sbuf.tile([128, 128], bf, name="stg_tpb", tag="stgTPB")
                        nc.scalar.copy(stg[:], pst[:])
                        nc.sync.dma_start(zuT[1][:, j * 128:(j + 1) * 128], stg[:])
                    else:
                        nc.sync.dma_start(zu_out[j * 128:(j + 1) * 128, :],
                                          zu_new[:, 0, :])

                edge_phase(blkB, KB, tbl[("xli", l)], tbl[("xru", l)],
                           srcB_t, xrB_t, ldB_t, ab_t[(l, 1)], flushB)

    nc.compile()
    return nc


# ----------------------------------------------------------------------------
# host wrapper
# ----------------------------------------------------------------------------

def _bcast_row(v, dtype):
    return np.ascontiguousarray(
        np.tile(np.asarray(v, F32).reshape(1, -1), (128, 1)).astype(dtype))


def prep_in_maps(inputs, cfg, cores):
    NC, USH, NIP = cfg["NC"], cfg["USH"], cfg["NIP"]
    NU, NI = cfg["NU"], cfg["NI"]
    x_user = np.asarray(inputs["x_user"], F32)
    x_item = np.asarray(inputs["x_item"], F32)
    xu_pad = np.zeros((NC * USH, 64), F32)
    xu_pad[:NU] = x_user
    xi_pad = np.zeros((NIP, 128), F32)
    xi_pad[:NI] = x_item
    xiT = np.ascontiguousarray(xi_pad.T.astype(BF16))

    Wl, bl = np.asarray(inputs["Wl"], F32), np.asarray(inputs["bl"], F32)
    Wr, br = np.asarray(inputs["Wr"], F32), np.asarray(inputs["br"], F32)
    att, obias = np.asarray(inputs["att"], F32), np.asarray(inputs["bias"], F32)

    shared = {
        "xiT": xiT,
        "wpu": np.asarray(inputs["Wp_user"], F32).astype(BF16),
        "wpi": np.asarray(inputs["Wp_item"], F32).astype(BF16),
        "bpu": np.asarray(inputs["bp_user"], F32).reshape(128, 1),
        "bpi": np.asarray(inputs["bp_item"], F32).reshape(128, 1),
        "iota": _bcast_row(np.arange(128), BF16),
    }
    for l in range(2):
        for r in range(2):
            shared[f"wl{l}{r}"] = Wl[l, r].astype(BF16)
            shared[f"wr{l}{r}"] = Wr[l, r].astype(BF16)
            shared[f"blb{l}{r}"] = _bcast_row(bl[l, r], F32)
            shared[f"brb{l}{r}"] = _bcast_row(br[l, r], F32)
            shared[f"ab{l}{r}"] = _bcast_row(att[l, r].reshape(128), BF16)
            shared[f"ob{l}{r}"] = _bcast_row(obias[l, r], F32)

    in_maps = []
    for c in range(NC):
        m = dict(shared)
        m["xuT"] = np.ascontiguousarray(
            xu_pad[c * USH:(c + 1) * USH].T.astype(BF16))
        m.update(cores[c])
        in_maps.append(m)
    return in_maps


def bias_flags(inputs):
    return dict(
        bp=bool(np.any(np.asarray(inputs["bp_user"])) or np.any(np.asarray(inputs["bp_item"]))),
        tab=bool(np.any(np.asarray(inputs["bl"])) or np.any(np.asarray(inputs["br"]))),
        out=bool(np.any(np.asarray(inputs["bias"]))),
    )


_BUILT = {}
LAST_RESULTS = None


def kernel(x_user, x_item, Wp_user, bp_user, Wp_item, bp_item,
           Wl, bl, Wr, br, att, bias, edge_src, edge_dst,
           trace=False):
    global LAST_RESULTS
    from concourse.bass_utils import run_bass_kernel_spmd

    cfg = FULL_CFG
    inputs = dict(x_user=x_user, x_item=x_item, Wp_user=Wp_user,
                  bp_user=bp_user, Wp_item=Wp_item, bp_item=bp_item,
                  Wl=Wl, bl=bl, Wr=Wr, br=br, att=att, bias=bias)
    eu = np.asarray(edge_src, np.int64)
    ei = np.asarray(edge_dst, np.int64)
    KA, KB, cores = plan(eu, ei, cfg)
    nz = bias_flags(inputs)

    key = (tuple(KA), tuple(KB), tuple(sorted(nz.items())))
    if key not in _BUILT:
        _BUILT.clear()
        _BUILT[key] = build(cfg, KA, KB, nz)
    nc = _BUILT[key]

    in_maps = prep_in_maps(inputs, cfg, cores)
    res = run_bass_kernel_spmd(nc, in_maps, core_ids=list(range(cfg["NC"])),
                               trace=trace)
    LAST_RESULTS = res
    zu = np.concatenate([res.results[c]["zu_out"] for c in range(cfg["NC"])],
                        axis=0)[:cfg["NU"]]
    zi = res.results[0]["zi_out"][:cfg["NI"]]
    return zu.astype(np.float32), zi.astype(np.float32)


# revision 8
# speedup vs baseline: 2.0388x; 1.1999x over previous
"""Trainium2 Bass kernel for 2-layer bipartite GATv2 (users <-> items).

Strategy (8 NeuronCores):
  * Edges are assigned to cores by their USER node (u // USH). Each core owns a
    contiguous user shard of USH=12544 users (100352 padded total).
  * u->i phase (rel 0): edge-parallel. Each core gathers source features from
    its LOCAL xl_u table (<=12544 rows, fits int16 gather indices), scatters
    per-edge messages into a full-item accumulator [20480, 130] via one-hot
    matmuls in PSUM, then one AllReduce(add) combines partial item sums.
  * i->u phase (rel 1): destination-sharded. Each core owns its user blocks;
    gathers source features from the full item table (20480 rows, int16-safe);
    no collective needed.
  * Per-edge pipeline in slabs of G=8 tiles (128 edges each), all bf16,
    engineered around DVE perf modes (no per-partition "scalar ptr" ops --
    those run ~1.1us each; broadcast-AP tensor_tensor instead):
      XS = dma_gather(xl_tbl, src)   XR = dma_gather(xr_tbl, dst)  (4 queues)
      T = XS + XR;  LR = max(0.2*T, T)
      logit = reduce_X(LR * a_bcast); e = exp(logit) -> mm[:,:,128:130]
      mm[:,:,h*64:(h+1)*64] = XS_h * e_h (broadcast)
      P[e,t,d] = (ldst[e,t] == iota[d]); psum[block] += P_t.T @ mm_t
  * Softmax max-subtraction is skipped (logits are O(0.1) at this init scale;
    fp32 exp cannot overflow and the 1e-16 epsilon stays negligible).
  * All-zero bias vectors (this problem) skip their add instructions.
"""
import sys

for _p in ("/opt/trn_rl_repo", "/opt/pypackages"):
    if _p not in sys.path:
        sys.path.insert(0, _p)

import numpy as np
import ml_dtypes

import concourse.bacc as bacc
import concourse.bass as bass
import concourse.tile as tile
from concourse import mybir, library_config
from concourse.masks import make_identity

BF16 = ml_dtypes.bfloat16
F32 = np.float32
ALPHA = 0.2  # leaky relu slope
EPS = 1e-16

FULL_CFG = dict(NU=100000, NI=20000, E=250000, NC=8, USH=12544, NIP=20480, G=8)


# ----------------------------------------------------------------------------
# host-side planning
# ----------------------------------------------------------------------------

def _wrap_idxs(idx: np.ndarray) -> np.ndarray:
    """dma_gather int16 index layout: [128, n/16], j -> [j%16, j//16], x8 replicas."""
    n = idx.shape[0]
    assert n % 16 == 0
    a = np.empty((16, n // 16), np.int16)
    a[np.arange(n) % 16, np.arange(n) // 16] = idx.astype(np.int16)
    return np.tile(a, (8, 1))


def _slot_fill(key, Ks, vals_dummies):
    """Place per-edge values into padded per-block tile slots."""
    nslot = int(Ks.sum()) * 128
    offs = np.zeros(len(Ks) + 1, np.int64)
    offs[1:] = np.cumsum(Ks * 128)
    order = np.argsort(key, kind="stable")
    sk = key[order]
    block_start = np.searchsorted(sk, np.arange(len(Ks)))
    rank = np.arange(len(sk)) - block_start[sk]
    pos = offs[sk] + rank
    outs = []
    for vals, dummy, dt in vals_dummies:
        a = np.full(nslot, dummy, dt)
        a[pos] = vals[order].astype(dt)
        outs.append(a)
    return outs


def plan(edge_u, edge_i, cfg):
    """Build the global tile schedules (KA, KB) and per-core edge arrays."""
    NC, USH, NIP = cfg["NC"], cfg["USH"], cfg["NIP"]
    NBI, NBU = NIP // 128, USH // 128
    owner = edge_u // USH
    per_core = []
    cntA = np.zeros((NC, NBI), np.int64)
    cntB = np.zeros((NC, NBU), np.int64)
    for c in range(NC):
        m = owner == c
        eu = edge_u[m] - c * USH
        ei = edge_i[m]
        cntA[c] = np.bincount(ei // 128, minlength=NBI)
        cntB[c] = np.bincount(eu // 128, minlength=NBU)
        per_core.append((eu, ei))
    KA = np.maximum(1, -(-cntA.max(0) // 128)).astype(np.int64)
    KB = np.maximum(1, -(-cntB.max(0) // 128)).astype(np.int64)

    cores = []
    for c in range(NC):
        eu, ei = per_core[c]
        srcA, xrA, ldA = _slot_fill(ei // 128, KA, [
            (eu, 0, np.int16), (ei, 0, np.int16), (ei % 128, 200, np.int16)])
        srcB, xrB, ldB = _slot_fill(eu // 128, KB, [
            (ei, 0, np.int16), (eu, 0, np.int16), (eu % 128, 200, np.int16)])
        NTA, NTB = int(KA.sum()), int(KB.sum())
        cores.append(dict(
            srcA=_wrap_idxs(srcA), xrA=_wrap_idxs(xrA),
            ldA=np.ascontiguousarray(ldA.reshape(NTA, 128).T.astype(BF16)),
            srcB=_wrap_idxs(srcB), xrB=_wrap_idxs(xrB),
            ldB=np.ascontiguousarray(ldB.reshape(NTB, 128).T.astype(BF16)),
        ))
    return KA, KB, cores


# ----------------------------------------------------------------------------
# kernel builder
# ----------------------------------------------------------------------------

def build(cfg, KA, KB, nz):
    """nz: dict of which bias groups are nonzero:
    keys 'bp', 'tab' (bl/br), 'out' (output bias)."""
    NC, USH, NIP, G = cfg["NC"], cfg["USH"], cfg["NIP"], cfg["G"]
    NBI, NBU = NIP // 128, USH // 128
    NTA, NTB = int(KA.sum()), int(KB.sum())
    blkA = np.repeat(np.arange(NBI), KA)
    blkB = np.repeat(np.arange(NBU), KB)
    bf = mybir.dt.bfloat16
    f32 = mybir.dt.float32
    i16 = mybir.dt.int16

    nc = bacc.Bacc("TRN2", num_devices=NC, num_swdge_queues=4)

    xuT = nc.dram_tensor("xuT", [64, USH], bf, kind="ExternalInput")
    xiT = nc.dram_tensor("xiT", [128, NIP], bf, kind="ExternalInput")
    wpu = nc.dram_tensor("wpu", [64, 128], bf, kind="ExternalInput")
    wpi = nc.dram_tensor("wpi", [128, 128], bf, kind="ExternalInput")
    bpu = nc.dram_tensor("bpu", [128, 1], f32, kind="ExternalInput")
    bpi = nc.dram_tensor("bpi", [128, 1], f32, kind="ExternalInput")
    iota_in = nc.dram_tensor("iota", [128, 128], bf, kind="ExternalInput")
    w_in, bb_in, ab_in, ob_in = {}, {}, {}, {}
    for l in range(2):
        for r in range(2):
            w_in[("l", l, r)] = nc.dram_tensor(f"wl{l}{r}", [128, 128], bf, kind="ExternalInput")
            w_in[("r", l, r)] = nc.dram_tensor(f"wr{l}{r}", [128, 128], bf, kind="ExternalInput")
            bb_in[("l", l, r)] = nc.dram_tensor(f"blb{l}{r}", [128, 128], f32, kind="ExternalInput")
            bb_in[("r", l, r)] = nc.dram_tensor(f"brb{l}{r}", [128, 128], f32, kind="ExternalInput")
            ab_in[(l, r)] = nc.dram_tensor(f"ab{l}{r}", [128, 128], bf, kind="ExternalInput")
            ob_in[(l, r)] = nc.dram_tensor(f"ob{l}{r}", [128, 128], f32, kind="ExternalInput")
    srcA_in = nc.dram_tensor("srcA", [128, NTA * 8], i16, kind="ExternalInput")
    xrA_in = nc.dram_tensor("xrA", [128, NTA * 8], i16, kind="ExternalInput")
    ldA_in = nc.dram_tensor("ldA", [128, NTA], bf, kind="ExternalInput")
    srcB_in = nc.dram_tensor("srcB", [128, NTB * 8], i16, kind="ExternalInput")
    xrB_in = nc.dram_tensor("xrB", [128, NTB * 8], i16, kind="ExternalInput")
    ldB_in = nc.dram_tensor("ldB", [128, NTB], bf, kind="ExternalInput")
    zu_out = nc.dram_tensor("zu_out", [USH, 128], f32, kind="ExternalOutput")
    zi_out = nc.dram_tensor("zi_out", [NIP, 128], f32, kind="ExternalOutput")

    AluOp = mybir.AluOpType
    ActFn = mybir.ActivationFunctionType
    qn = [0]

    with tile.TileContext(nc, num_cores=NC) as tc:
        with (
            tc.tile_pool(name="const", bufs=1) as const,
            tc.tile_pool(name="sbuf", bufs=2) as sbuf,
            tc.tile_pool(name="psum", bufs=2, space="PSUM") as psum,
            tc.tile_pool(name="dram", bufs=1, space="DRAM") as dram,
        ):
            nc.gpsimd.load_library(library_config.mlp)

            def cload(src, shape, dtype, name):
                t = const.tile(shape, dtype, name=name, tag=name)
                nc.sync.dma_start(t[:], src[:])
                return t

            iota_t = cload(iota_in, [128, 128], bf, "iota_t")
            wpu_t = cload(wpu, [64, 128], bf, "wpu_t")
            wpi_t = cload(wpi, [128, 128], bf, "wpi_t")
            bpu_t = cload(bpu, [128, 1], f32, "bpu_t")
            bpi_t = cload(bpi, [128, 1], f32, "bpi_t")
            w_t, bb_t, ab_t, ob_t = {}, {}, {}, {}
            for l in range(2):
                for r in range(2):
                    for s in ("l", "r"):
                        w_t[(s, l, r)] = cload(w_in[(s, l, r)], [128, 128], bf, f"w{s}{l}{r}_t")
                        if nz["tab"]:
                            bb_t[(s, l, r)] = cload(bb_in[(s, l, r)], [128, 128], f32, f"b{s}b{l}{r}_t")
                    ab_t[(l, r)] = cload(ab_in[(l, r)], [128, 128], bf, f"ab{l}{r}_t")
                    if nz["out"]:
                        ob_t[(l, r)] = cload(ob_in[(l, r)], [128, 128], f32, f"ob{l}{r}_t")
            srcA_t = cload(srcA_in, [128, NTA * 8], i16, "srcA_t")
            xrA_t = cload(xrA_in, [128, NTA * 8], i16, "xrA_t")
            ldA_t = cload(ldA_in, [128, NTA], bf, "ldA_t")
            srcB_t = cload(srcB_in, [128, NTB * 8], i16, "srcB_t")
            xrB_t = cload(xrB_in, [128, NTB * 8], i16, "xrB_t")
            ldB_t = cload(ldB_in, [128, NTB], bf, "ldB_t")
            ident_t = const.tile([128, 128], f32, name="ident_t", tag="ident_t")
            make_identity(nc, ident_t[:])

            def dtile(name, shape, dtype, shared=False):
                return dram.tile(shape, dtype, name=name, tag=name,
                                 addr_space="Shared" if shared else "Local")

            zuT = [dtile(f"zuT{i}", [128, USH], bf) for i in range(2)]
            ziT = [dtile(f"ziT{i}", [128, NIP], bf) for i in range(2)]
            tbl = {}
            for l in range(2):
                for nm in ("xlu", "xru"):
                    tbl[(nm, l)] = dtile(f"{nm}{l}", [USH, 128], bf)
                for nm in ("xli", "xri"):
                    tbl[(nm, l)] = dtile(f"{nm}{l}", [NIP, 128], bf)
            accA = [dtile(f"accA{l}", [NIP, 130], f32) for l in range(2)]
            accAr = [dtile(f"accAr{l}", [NIP, 130], f32, shared=True) for l in range(2)]

            # ---- initial projections (feature-major out) --------------------
            def init_proj(xT, w_tile, b_col, dstT, ncols, kdim):
                for c0 in range(0, ncols, 512):
                    w = min(512, ncols - c0)
                    xc = sbuf.tile([128, 512], bf, name="xc_init", tag="xcI")
                    nc.sync.dma_start(xc[:kdim, :w], xT[:kdim, c0:c0 + w])
                    ps = psum.tile([128, 512], f32, name="ps_init", tag="psD")
                    nc.tensor.matmul(ps[:, :w], lhsT=w_tile[:kdim, :], rhs=xc[:kdim, :w],
                                     start=True, stop=True)
                    stg = sbuf.tile([128, 512], bf, name="stg_init", tag="stgD")
                    if nz["bp"]:
                        nc.scalar.activation(stg[:, :w], ps[:, :w], ActFn.Identity,
                                             bias=b_col[:])
                    else:
                        nc.scalar.copy(stg[:, :w], ps[:, :w])
                    nc.sync.dma_start(dstT[:, c0:c0 + w], stg[:, :w])

            init_proj(xuT, wpu_t, bpu_t, zuT[0], USH, 64)
            init_proj(xiT, wpi_t, bpi_t, ziT[0], NIP, 128)

            # ---- per node-tile linear tables (4 tiles per DMA chunk) -------
            def make_tables(zT, nblocks, wA, bA, dstA, wB, bB, dstB):
                for n0 in range(0, nblocks, 4):
                    nb = min(4, nblocks - n0)
                    zt = sbuf.tile([128, 4 * 128], bf, name="zt_d", tag="ztD")
                    nc.sync.dma_start(zt[:, :nb * 128], zT[:, n0 * 128:(n0 + nb) * 128])
                    for w_tile, b_tile, dst in ((wA, bA, dstA), (wB, bB, dstB)):
                        stg = sbuf.tile([128, 4, 128], bf, name="stg_tab", tag="stgT")
                        for k in range(nb):
                            ps = psum.tile([128, 512], f32, name="ps_tab", tag="psD")
                            nc.tensor.matmul(ps[:, :128], lhsT=zt[:, k * 128:(k + 1) * 128],
                                             rhs=w_tile[:], start=True, stop=True)
                            if b_tile is not None:
                                nc.vector.tensor_tensor(out=stg[:, k, :], in0=ps[:, :128],
                                                        in1=b_tile[:], op=AluOp.add)
                            else:
                                nc.scalar.copy(stg[:, k, :], ps[:, :128])
                        nc.sync.dma_start(
                            dst[n0 * 128:(n0 + nb) * 128, :].rearrange(
                                "(t p) f -> p t f", p=128),
                            stg[:, :nb, :])

            # ---- edge phase -------------------------------------------------
            def edge_phase(tile_blk, Ks, src_tbl, xr_tbl, src_idx, xr_idx, ld_t,
                           avec, flush):
                NT = len(tile_blk)
                n_in_blk = 0
                ps = None
                iota_b = iota_t[:].rearrange("p (g f) -> p g f", g=1)
                avec_b = avec[:].rearrange("p (g f) -> p g f", g=1)
                for t0 in range(0, NT, G):
                    g = min(G, NT - t0)
                    ni = g * 128
                    xs = sbuf.tile([128, G, 128], bf, name="xs_e", tag="xsE", bufs=4)
                    xr = sbuf.tile([128, G, 128], bf, name="xr_e", tag="xrE", bufs=4)
                    nc.gpsimd.dma_gather(xs[:, :g, :], src_tbl[:],
                                         src_idx[:, t0 * 8:(t0 + g) * 8], ni, ni, 128,
                                         queue_num=qn[0] % 4)
                    qn[0] += 1
                    nc.gpsimd.dma_gather(xr[:, :g, :], xr_tbl[:],
                                         xr_idx[:, t0 * 8:(t0 + g) * 8], ni, ni, 128,
                                         queue_num=qn[0] % 4)
                    qn[0] += 1
                    tt = sbuf.tile([128, G, 128], bf, name="tt_e", tag="ttE", bufs=3)
                    nc.vector.tensor_tensor(out=tt[:, :g, :], in0=xs[:, :g, :],
                                            in1=xr[:, :g, :], op=AluOp.add)
                    lr = sbuf.tile([128, G, 128], bf, name="lr_e", tag="lrE", bufs=3)
                    nc.vector.scalar_tensor_tensor(
                        out=lr[:, :g, :], in0=tt[:, :g, :], scalar=ALPHA,
                        in1=tt[:, :g, :], op0=AluOp.mult, op1=AluOp.max)
                    uu = sbuf.tile([128, G, 128], bf, name="uu_e", tag="uuE")
                    nc.vector.tensor_tensor(
                        out=uu[:, :g, :], in0=lr[:, :g, :],
                        in1=avec_b.to_broadcast([128, g, 128]), op=AluOp.mult)
                    lg = sbuf.tile([128, G, 2], f32, name="lg_e", tag="lgE")
                    nc.vector.tensor_reduce(
                        out=lg[:, :g, :],
                        in_=uu[:, :g, :].rearrange("p g (h d) -> p g h d", h=2),
                        axis=mybir.AxisListType.X, op=AluOp.add)
                    mm = sbuf.tile([128, G, 130], bf, name="mm_e", tag="mmE", bufs=3)
                    nc.scalar.activation(mm[:, :g, 128:130], lg[:, :g, :], ActFn.Exp)
                    for h in range(2):
                        nc.vector.tensor_tensor(
                            out=mm[:, :g, h * 64:(h + 1) * 64],
                            in0=xs[:, :g, h * 64:(h + 1) * 64],
                            in1=mm[:, :g, 128 + h:129 + h].to_broadcast([128, g, 64]),
                            op=AluOp.mult)
                    pp = sbuf.tile([128, G, 128], bf, name="pp_e", tag="ppE", bufs=3)
                    nc.vector.tensor_tensor(
                        out=pp[:, :g, :],
                        in0=iota_b.to_broadcast([128, g, 128]),
                        in1=ld_t[:, t0:t0 + g].to_broadcast([128, g, 128]),
                        op=AluOp.is_equal)
                    for k in range(g):
                        t = t0 + k
                        b = tile_blk[t]
                        if n_in_blk == 0:
                            ps = psum.tile([128, 130], f32, name="ps_e", tag="psE",
                                           bufs=3)
                        first = n_in_blk == 0
                        last = n_in_blk == Ks[b] - 1
                        nc.tensor.matmul(ps[:, 0:130], lhsT=pp[:, k, :], rhs=mm[:, k, :],
                                         start=first, stop=last)
                        if last:
                            flush(b, ps)
                            n_in_blk = 0
                        else:
                            n_in_blk += 1

            # ---- normalize: out = msg/(den+eps) [+bias] [relu] -------------
            def normalize(acc3, nb, obias, do_relu, out3):
                """acc3: [128, nb, 130] AP (psum or sbuf); out3: [128, nb, 128] f32."""
                dpe = sbuf.tile([128, 4, 2], f32, name="dpe_n", tag="dpeN")
                nc.vector.tensor_scalar_add(dpe[:, :nb, :], acc3[:, :, 128:130], EPS)
                rcp = sbuf.tile([128, 4, 2], f32, name="rcp_n", tag="rcpN")
                nc.vector.reciprocal(rcp[:, :nb, :], dpe[:, :nb, :])
                for h in range(2):
                    nc.vector.tensor_tensor(
                        out=out3[:, :, h * 64:(h + 1) * 64],
                        in0=acc3[:, :, h * 64:(h + 1) * 64],
                        in1=rcp[:, :nb, h:h + 1].to_broadcast([128, nb, 64]),
                        op=AluOp.mult)
                if obias is not None:
                    nc.vector.tensor_tensor(
                        out=out3[:], in0=out3[:],
                        in1=obias[:].rearrange("p (g f) -> p g f", g=1)
                        .to_broadcast([128, nb, 128]),
                        op=AluOp.add)
                if do_relu:
                    nc.vector.tensor_scalar_max(out3[:], out3[:], 0.0)

            # ---- layers -----------------------------------------------------
            for l in range(2):
                bbt = (lambda s, r: bb_t[(s, l, r)]) if nz["tab"] else (lambda s, r: None)
                obt = (lambda r: ob_t[(l, r)]) if nz["out"] else (lambda r: None)
                make_tables(zuT[l], NBU,
                            w_t[("l", l, 0)], bbt("l", 0), tbl[("xlu", l)],
                            w_t[("r", l, 1)], bbt("r", 1), tbl[("xru", l)])
                make_tables(ziT[l], NBI,
                            w_t[("r", l, 0)], bbt("r", 0), tbl[("xri", l)],
                            w_t[("l", l, 1)], bbt("l", 1), tbl[("xli", l)])

                # ---- A phase: u -> i (edge parallel + AllReduce) ----------
                def flushA(b, ps, l=l):
                    stg = sbuf.tile([128, 130], f32, name="stg_fa", tag="stgFA")
                    nc.scalar.copy(stg[:], ps[:])
                    nc.sync.dma_start(accA[l][b * 128:(b + 1) * 128, :], stg[:])

                edge_phase(blkA, KA, tbl[("xlu", l)], tbl[("xri", l)],
                           srcA_t, xrA_t, ldA_t, ab_t[(l, 0)], flushA)
                nc.gpsimd.collective_compute(
                    "AllReduce", AluOp.add,
                    replica_groups=[list(range(NC))],
                    ins=[accA[l][:].opt()], outs=[accAr[l][:].opt()])

                # ---- post A: zi_new (4 blocks per batch) ------------------
                for b0 in range(0, NBI, 4):
                    nb = min(4, NBI - b0)
                    acc = sbuf.tile([128, 4, 130], f32, name="acc_pa", tag="accPA")
                    nc.sync.dma_start(
                        acc[:, :nb, :],
                        accAr[l][b0 * 128:(b0 + nb) * 128, :].rearrange(
                            "(t p) c -> p t c", p=128))
                    zi_new = sbuf.tile([128, 4, 128], f32, name="zin_pa", tag="zinPA")
                    normalize(acc[:, :nb, :], nb, obt(0), l == 0, zi_new[:, :nb, :])
                    if l == 0:
                        stg = sbuf.tile([128, 4 * 128], bf, name="stg_tp", tag="stgTP")
                        for k in range(nb):
                            pst = psum.tile([128, 128], f32, name="ps_tp", tag="psT")
                            nc.tensor.transpose(pst[:], zi_new[:, k, :], ident_t[:])
                            nc.scalar.copy(stg[:, k * 128:(k + 1) * 128], pst[:])
                        nc.sync.dma_start(ziT[1][:, b0 * 128:(b0 + nb) * 128],
                                          stg[:, :nb * 128])
                    else:
                        nc.sync.dma_start(
                            zi_out[b0 * 128:(b0 + nb) * 128, :].rearrange(
                                "(t p) f -> p t f", p=128),
                            zi_new[:, :nb, :])

                # ---- B phase: i -> u (dst sharded, local) -----------------
                def flushB(j, ps, l=l):
                    zu_new = sbuf.tile([128, 1, 128], f32, name="zun_fb", tag="zunFB")
                    ps3 = ps[:].rearrange("p (g c) -> p g c", g=1)
                    normalize(ps3, 1, obt(1), l == 0, zu_new[:, :1, :])
                    if l == 0:
                        pst = psum.tile([128, 128], f32, name="ps_tpb", tag="psT")
                        nc.tensor.transpose(pst[:], zu_new[:, 0, :], ident_t[:])
                        stg = sbuf.tile([128, 128], bf, name="stg_tpb", tag="stgTPB")
                        nc.scalar.copy(stg[:], pst[:])
                        nc.sync.dma_start(zuT[1][:, j * 128:(j + 1) * 128], stg[:])
                    else:
                        nc.sync.dma_start(zu_out[j * 128:(j + 1) * 128, :],
                                          zu_new[:, 0, :])

                edge_phase(blkB, KB, tbl[("xli", l)], tbl[("xru", l)],
                           srcB_t, xrB_t, ldB_t, ab_t[(l, 1)], flushB)

    nc.compile()
    return nc


# ----------------------------------------------------------------------------
# host wrapper
# ----------------------------------------------------------------------------

def _bcast_row(v, dtype):
    return np.ascontiguousarray(
        np.tile(np.asarray(v, F32).reshape(1, -1), (128, 1)).astype(dtype))


def prep_in_maps(inputs, cfg, cores):
    NC, USH, NIP = cfg["NC"], cfg["USH"], cfg["NIP"]
    NU, NI = cfg["NU"], cfg["NI"]
    x_user = np.asarray(inputs["x_user"], F32)
    x_item = np.asarray(inputs["x_item"], F32)
    xu_pad = np.zeros((NC * USH, 64), F32)
    xu_pad[:NU] = x_user
    xi_pad = np.zeros((NIP, 128), F32)
    xi_pad[:NI] = x_item
    xiT = np.ascontiguousarray(xi_pad.T.astype(BF16))

    Wl, bl = np.asarray(inputs["Wl"], F32), np.asarray(inputs["bl"], F32)
    Wr, br = np.asarray(inputs["Wr"], F32), np.asarray(inputs["br"], F32)
    att, obias = np.asarray(inputs["att"], F32), np.asarray(inputs["bias"], F32)

    shared = {
        "xiT": xiT,
        "wpu": np.asarray(inputs["Wp_user"], F32).astype(BF16),
        "wpi": np.asarray(inputs["Wp_item"], F32).astype(BF16),
        "bpu": np.asarray(inputs["bp_user"], F32).reshape(128, 1),
        "bpi": np.asarray(inputs["bp_item"], F32).reshape(128, 1),
        "iota": _bcast_row(np.arange(128), BF16),
    }
    for l in range(2):
        for r in range(2):
            shared[f"wl{l}{r}"] = Wl[l, r].astype(BF16)
            shared[f"wr{l}{r}"] = Wr[l, r].astype(BF16)
            shared[f"blb{l}{r}"] = _bcast_row(bl[l, r], F32)
            shared[f"brb{l}{r}"] = _bcast_row(br[l, r], F32)
            shared[f"ab{l}{r}"] = _bcast_row(att[l, r].reshape(128), BF16)
            shared[f"ob{l}{r}"] = _bcast_row(obias[l, r], F32)

    in_maps = []
    for c in range(NC):
        m = dict(shared)
        m["xuT"] = np.ascontiguousarray(
            xu_pad[c * USH:(c + 1) * USH].T.astype(BF16))
        m.update(cores[c])
        in_maps.append(m)
    return in_maps


def bias_flags(inputs):
    return dict(
        bp=bool(np.any(np.asarray(inputs["bp_user"])) or np.any(np.asarray(inputs["bp_item"]))),
        tab=bool(np.any(np.asarray(inputs["bl"])) or np.any(np.asarray(inputs["br"]))),
        out=bool(np.any(np.asarray(inputs["bias"]))),
    )


_BUILT = {}
LAST_RESULTS = None


def kernel(x_user, x_item, Wp_user, bp_user, Wp_item, bp_item,
           Wl, bl, Wr, br, att, bias, edge_src, edge_dst,
           trace=False):
    global LAST_RESULTS
    from concourse.bass_utils import run_bass_kernel_spmd

    cfg = FULL_CFG
    inputs = dict(x_user=x_user, x_item=x_item, Wp_user=Wp_user,
                  bp_user=bp_user, Wp_item=Wp_item, bp_item=bp_item,
                  Wl=Wl, bl=bl, Wr=Wr, br=br, att=att, bias=bias)
    eu = np.asarray(edge_src, np.int64)
    ei = np.asarray(edge_dst, np.int64)
    KA, KB, cores = plan(eu, ei, cfg)
    nz = bias_flags(inputs)

    key = (tuple(KA), tuple(KB), tuple(sorted(nz.items())))
    if key not in _BUILT:
        _BUILT.clear()
        _BUILT[key] = build(cfg, KA, KB, nz)
    nc = _BUILT[key]

    in_maps = prep_in_maps(inputs, cfg, cores)
    res = run_bass_kernel_spmd(nc, in_maps, core_ids=list(range(cfg["NC"])),
                               trace=trace)
    LAST_RESULTS = res
    zu = np.concatenate([res.results[c]["zu_out"] for c in range(cfg["NC"])],
                        axis=0)[:cfg["NU"]]
    zi = res.results[0]["zi_out"][:cfg["NI"]]
    return zu.astype(np.float32), zi.astype(np.float32)
